# revision 1
# baseline (speedup 1.0000x reference)
"""Causal self-attention Trainium2 kernel (8-core SPMD).

Problem: x[2,2048,1024], causal mask, Wqkv[3072,1024], Wo[1024,1024], fp32.
  qkv = x @ Wqkv.T ; per-head causal softmax attention ; out = attn @ Wo.T

Sharding (data + tensor parallel, per the head dimension):
  core c -> batch b = c // 4, heads {4g..4g+3} with g = c % 4.
  Each core computes Q,K,V for its 4 heads (512 qk cols + 256 v cols of the
  projection), runs causal attention for those heads, and multiplies by the
  matching 256 columns of Wo, producing a partial [2048, 1024] output.
  Host sums the 4 partials per batch (the tensor-parallel reduction).

Kernel structure (per core):
  - bf16 matmul operands (PE 1 cyc/row), fp32 PSUM accumulation.
  - Projection chunks (ko-outer, so PE starts as soon as the first 128-row
    slices of x/w arrive) are interleaved with attention chunks: attention
    for q-chunk qc needs only projection chunks nn <= qc, so ACT exp work
    overlaps PE projection matmuls.
  - Scores are computed TRANSPOSED (scoresT[k, q], head pairs packed in the
    PE via partition-base row tiling) so AV needs no transposes. Score
    blocks go into 2-bank PSUM tiles (two k-blocks per tile) so one
    ACTIVATE exps 1024 columns, halving ACT instruction overhead.
  - Causality: strictly-upper blocks skipped; diagonal straddlers compute
    only the valid columns; the 128x128 diagonal sub-block is exp'd
    unmasked then multiplied by a binary mask tile (from the mask input).
  - V carries a ones column (65 cols/head): AV's partition 64 accumulates
    the softmax denominator for free. Normalization = fast-approx
    reciprocal (sums >= 1), broadcast over partitions via a K=1
    ones-matmul, one DVE multiply.
"""

import os

import numpy as np

S = 2048
D = 1024
DH = 64
B = 2
NCORES = 8
HPC = 4  # heads per core
QKC = 2 * HPC * DH  # 512 q+k projection columns per core
VC = HPC * DH  # 256 v columns per core
P = 128
KO = D // P  # 8 contraction tiles
NQ = S // 512  # 4 q-chunks of 512
NSC = S // P  # 16 s-chunks of 128

COMPUTE_DT = os.environ.get("ATTN_COMPUTE_DT", "bf16")  # bf16 | f32r

_cache = {}


def _np_compute_dt():
    if COMPUTE_DT == "bf16":
        import ml_dtypes

        return ml_dtypes.bfloat16
    return np.float32


def _build():
    import concourse.bacc as bacc
    import concourse.mybir as mybir
    import concourse.tile as tile

    F32 = mybir.dt.float32
    CDT = mybir.dt.bfloat16 if COMPUTE_DT == "bf16" else mybir.dt.float32r
    EXP = mybir.ActivationFunctionType.Exp

    nc = bacc.Bacc()
    xT_d = nc.dram_tensor("xT", [D, S], CDT, kind="ExternalInput")
    wqkT_d = nc.dram_tensor("wqkT", [D, QKC], CDT, kind="ExternalInput")
    wvT_d = nc.dram_tensor("wvT", [D, VC], CDT, kind="ExternalInput")
    woT_d = nc.dram_tensor("woT", [VC, D], CDT, kind="ExternalInput")
    maskT_d = nc.dram_tensor("maskT", [P, P], CDT, kind="ExternalInput")
    out_d = nc.dram_tensor("out", [S, D], F32, kind="ExternalOutput")

    with tile.TileContext(nc) as tc:
        with (
            tc.tile_pool(name="persist", bufs=1) as persist,
            tc.tile_pool(name="sb_small", bufs=3) as sb_small,
            tc.tile_pool(name="sb_exp", bufs=12) as sb_exp,
            tc.tile_pool(name="sb_out", bufs=3) as sb_out,
            tc.tile_pool(name="pp_big", bufs=2, space="PSUM") as pp_big,
            tc.tile_pool(name="pp_av", bufs=2, space="PSUM") as pp_av,
            tc.tile_pool(name="pp_o", bufs=2, space="PSUM") as pp_o,
        ):
            xT_sb = persist.tile([P, KO, S], CDT, tag="xT")
            wqkT_sb = persist.tile([P, KO, QKC], CDT, tag="wqkT")
            wvT_sb = persist.tile([P, KO, VC], CDT, tag="wvT")
            woT_sb = persist.tile([P, 2, D], CDT, tag="woT")
            maskT_sb = persist.tile([P, P], CDT, tag="maskT")
            qkT_sb = persist.tile([P, 4, S], CDT, tag="qkT")
            v_sb = persist.tile([P, NSC, HPC, DH + 1], CDT, tag="v")
            attn_sb = persist.tile([P, 2, S], CDT, tag="attn")

            # --- input DMAs: wqkT[ko]+xT[ko] pairs first (unblock proj ko
            # chains asap), across two queues; then wvT, mask, woT ---
            for ko in range(KO):
                e1, e2 = (nc.sync, nc.gpsimd) if ko % 2 == 0 else (nc.gpsimd, nc.sync)
                e1.dma_start(wqkT_sb[:, ko, :], wqkT_d[ko * P : (ko + 1) * P, :])
                e2.dma_start(xT_sb[:, ko, :], xT_d[ko * P : (ko + 1) * P, :])
                e1.dma_start(wvT_sb[:, ko, :], wvT_d[ko * P : (ko + 1) * P, :])
            nc.sync.dma_start(maskT_sb[:], maskT_d[:])
            nc.gpsimd.dma_start(woT_sb[:], woT_d.rearrange("(ct p) e -> p ct e", p=P))
            ones_f32 = persist.tile([P, DH], F32, tag="ones_f32")
            nc.vector.memset(ones_f32[:], 1.0)
            nc.vector.tensor_copy(
                out=v_sb[:, :, :, DH],
                in_=ones_f32[:, 0 : NSC * HPC].rearrange("p (a b) -> p a b", a=NSC),
            )

            def emit_outproj(qc):
                for si in range(4):
                    sc = qc * 4 + si
                    for en in range(2):
                        ps_o = pp_o.tile([P, 512], F32, tag="o")
                        for ct in range(2):
                            nc.tensor.matmul(
                                ps_o[:],
                                attn_sb[:, ct, sc * P : (sc + 1) * P],
                                woT_sb[:, ct, en * 512 : (en + 1) * 512],
                                start=(ct == 0),
                                stop=(ct == 1),
                            )
                        o_sb = sb_out.tile([P, 512], F32, tag="osb")
                        nc.vector.tensor_copy(out=o_sb[:], in_=ps_o[:])
                        nc.sync.dma_start(
                            out_d[sc * P : (sc + 1) * P, en * 512 : (en + 1) * 512],
                            o_sb[:],
                        )

            for qc in range(NQ):
                # --- qk projection chunk nn = qc, ko-outer over two 2-bank
                # tiles (4 half-bank chains), so PE tracks DMA arrival ---
                pjA = pp_big.tile([P, 1024], F32, tag="big", name="pjA")
                pjB = pp_big.tile([P, 1024], F32, tag="big", name="pjB")
                for ko in range(KO):
                    for mm in range(4):
                        slot = pjA if mm < 2 else pjB
                        nc.tensor.matmul(
                            slot[:, (mm % 2) * 512 : (mm % 2 + 1) * 512],
                            wqkT_sb[:, ko, mm * P : (mm + 1) * P],
                            xT_sb[:, ko, qc * 512 : (qc + 1) * 512],
                            start=(ko == 0),
                            stop=(ko == KO - 1),
                            skip_group_check=True,
                        )
                nc.vector.tensor_copy(
                    out=qkT_sb[:, 0:2, qc * 512 : (qc + 1) * 512],
                    in_=pjA.rearrange("p (a b) -> p a b", a=2),
                )
                nc.vector.tensor_copy(
                    out=qkT_sb[:, 2:4, qc * 512 : (qc + 1) * 512],
                    in_=pjB.rearrange("p (a b) -> p a b", a=2),
                )

                # --- v projection for s-chunks 4qc..4qc+3 (4 bank chains) ---
                pvA = pp_big.tile([P, 1024], F32, tag="big", name="pvA")
                pvB = pp_big.tile([P, 1024], F32, tag="big", name="pvB")
                for ko in range(KO):
                    for j in range(4):
                        slot = pvA if j < 2 else pvB
                        sc = 4 * qc + j
                        nc.tensor.matmul(
                            slot[:, (j % 2) * 512 : (j % 2) * 512 + VC],
                            xT_sb[:, ko, sc * P : (sc + 1) * P],
                            wvT_sb[:, ko, :],
                            start=(ko == 0),
                            stop=(ko == KO - 1),
                            skip_group_check=True,
                        )
                for half, slot in ((0, pvA), (1, pvB)):
                    nc.vector.tensor_copy(
                        out=v_sb[:, 4 * qc + 2 * half : 4 * qc + 2 * half + 2, :, 0:DH],
                        in_=slot.rearrange("p (a h d) -> p a h d", a=2, h=8)[:, :, 0:HPC, :],
                    )

                # --- attention for q-chunk qc ---
                nkb = 4 * qc + 4  # causal: k blocks 0 .. 4qc+3
                for h in range(HPC):
                    hp = (h % 2) * DH  # partition base within the m-tile
                    mq = h // 2  # Q m-tile; K m-tile = 2 + h//2
                    avs = []
                    for kb0 in range(0, nkb, 2):
                        ps2 = pp_big.tile([P, 1024], F32, tag="big", name="ps2")
                        exp2 = sb_exp.tile([P, 1024], CDT, tag="exp")
                        offs = []
                        for half in (0, 1):
                            kb = kb0 + half
                            m = kb - 4 * qc  # >= 0 on diagonal straddlers
                            off = max(0, m) * P
                            offs.append(off)
                            nc.tensor.matmul(
                                ps2[:, half * 512 + off : half * 512 + 512],
                                qkT_sb[hp : hp + DH, 2 + mq, kb * P : (kb + 1) * P],
                                qkT_sb[hp : hp + DH, mq, qc * 512 + off : (qc + 1) * 512],
                                start=True,
                                stop=True,
                                skip_group_check=True,
                            )
                        if offs[0] == 0 and offs[1] == 0:
                            # dense pair: one 1024-wide exp
                            nc.scalar.activation(exp2[:], ps2[:], EXP, scale=0.125)
                        else:
                            for half, off in enumerate(offs):
                                lo = half * 512 + off
                                nc.scalar.activation(
                                    exp2[:, lo : half * 512 + 512],
                                    ps2[:, lo : half * 512 + 512],
                                    EXP,
                                    scale=0.125,
                                )
                        for half, off in enumerate(offs):
                            kb = kb0 + half
                            if kb - 4 * qc >= 0:
                                lo = half * 512 + off
                                nc.vector.tensor_mul(
                                    out=exp2[:, lo : lo + P],
                                    in0=exp2[:, lo : lo + P],
                                    in1=maskT_sb[:],
                                )
                            avs.append((exp2, half * 512 + off, off, kb))
                    ps_av = pp_av.tile([DH + 1, 512], F32, tag="av")
                    for j, (exp2, lo, off, kb) in enumerate(avs):
                        nc.tensor.matmul(
                            ps_av[:, off:512],
                            v_sb[:, kb, h, :],
                            exp2[:, lo : (lo - off) + 512],
                            start=(j == 0),
                            stop=(j == len(avs) - 1),
                            skip_group_check=True,
                        )
                    # normalize: out = av * (1/sums) broadcast over partitions
                    sums_sb = sb_small.tile([1, 512], F32, tag="sums")
                    nc.vector.tensor_copy(out=sums_sb[:], in_=ps_av[DH : DH + 1, :])
                    recip_f = sb_small.tile([1, 512], F32, tag="recipf")
                    nc.vector.reciprocal_approx_fast(out=recip_f[:], in_=sums_sb[:])
                    bc_sb = sb_small.tile([DH, 512], F32, tag="bc")
                    nc.gpsimd.partition_broadcast(bc_sb[:], recip_f[:])
                    nc.vector.tensor_mul(
                        out=attn_sb[hp : hp + DH, h // 2, qc * 512 : (qc + 1) * 512],
                        in0=ps_av[0:DH, :],
                        in1=bc_sb[:],
                    )

                # --- deferred output projection (previous q chunk) ---
                if qc > 0:
                    emit_outproj(qc - 1)
            emit_outproj(NQ - 1)

    nc.compile()
    return nc


def _get_nc():
    if "nc" not in _cache:
        _cache["nc"] = _build()
    return _cache["nc"]


def _shard(x, mask, Wqkv, Wo):
    cdt = _np_compute_dt()
    in_maps = []
    # binary mask for the transposed 128x128 diagonal block:
    # valid (mask==0) -> 1.0, masked (-inf/large-negative) -> 0.0
    maskT = np.ascontiguousarray((mask[0, 0, :P, :P].T >= 0).astype(cdt))
    for c in range(NCORES):
        b = c // 4
        g = c % 4
        heads = [4 * g + i for i in range(HPC)]
        q_rows = np.concatenate([np.arange(h * DH, (h + 1) * DH) for h in heads])
        k_rows = D + q_rows
        v_rows = 2 * D + q_rows
        qk_rows = np.concatenate([q_rows, k_rows])
        in_maps.append(
            {
                "xT": np.ascontiguousarray(x[b].T.astype(cdt)),
                "wqkT": np.ascontiguousarray(Wqkv[qk_rows, :].T.astype(cdt)),
                "wvT": np.ascontiguousarray(Wqkv[v_rows, :].T.astype(cdt)),
                "woT": np.ascontiguousarray(Wo[:, q_rows].T.astype(cdt)),
                "maskT": maskT,
            }
        )
    return in_maps


def kernel(x, mask, Wqkv, Wo, _trace=False):
    from concourse.bass_utils import run_bass_kernel_spmd

    x = np.asarray(x, dtype=np.float32)
    mask = np.asarray(mask, dtype=np.float32)
    Wqkv = np.asarray(Wqkv, dtype=np.float32)
    Wo = np.asarray(Wo, dtype=np.float32)

    nc = _get_nc()
    in_maps = _shard(x, mask, Wqkv, Wo)
    res = run_bass_kernel_spmd(nc, in_maps, core_ids=list(range(NCORES)), trace=_trace)
    _cache["last_result"] = res

    out = np.zeros((B, S, D), dtype=np.float32)
    for c in range(NCORES):
        out[c // 4] += res.results[c]["out"]
    return out



# revision 3
# speedup vs baseline: 1.0913x; 1.0913x over previous
"""Causal self-attention Trainium2 kernel (8-core SPMD).

Problem: x[2,2048,1024], causal mask, Wqkv[3072,1024], Wo[1024,1024], fp32.
  qkv = x @ Wqkv.T ; per-head causal softmax attention ; out = attn @ Wo.T

Sharding (data + tensor parallel, per the head dimension):
  core c -> batch b = c // 4, heads {4g..4g+3} with g = c % 4.
  Each core computes Q,K,V for its 4 heads (512 qk cols + 256 v cols of the
  projection), runs causal attention for those heads, and multiplies by the
  matching 256 columns of Wo, producing a partial [2048, 1024] output (bf16).
  Host sums the 4 partials per batch (the tensor-parallel reduction).

Kernel structure (per core):
  - bf16 matmul operands (PE 1 cyc/row), fp32 PSUM accumulation.
  - Projection chunks ko-outer so PE tracks DMA arrival; input DMA is ordered
    (wqkT[ko] + xT[ko, s-chunk0]) pairs first so the first q-chunk's
    projection completes after ~2MB instead of ~5MB of input traffic.
  - Attention per q-chunk runs in two head-pair passes. Scores are computed
    TRANSPOSED (scoresT[k, q]); the two heads of a pair occupy disjoint
    partition halves of qkT (hp 0 / 64), so their QK^T matmuls are issued
    back-to-back and run CONCURRENTLY in the PE via row tiling
    (tile_position (0,0) / (64,0)) - 2x QK throughput vs serial issue.
  - Head-pair score blocks land in one 2-bank PSUM tile ([h_even | h_odd]
    512 cols each) so one ACTIVATE exps 1024 columns.
  - Causality: strictly-upper blocks skipped; diagonal straddlers compute
    only the valid columns; the 128x128 diagonal sub-block is exp'd
    unmasked then multiplied by a binary mask tile (from the mask input).
  - V carries a ones column (65 cols/head): AV's partition 64 accumulates
    the softmax denominator for free. Normalization reads the denominator
    row straight from PSUM (fast-approx reciprocal, sums >= 1), broadcasts
    over partitions on gpsimd, one DVE multiply.
  - AV matmuls are emitted lagging the exp pipeline by 2 k-blocks so the
    PE never sits behind an in-flight ACTIVATE.
  - Output partials are cast to bf16 (DVE) and DMA'd out, halving output
    HBM traffic; the host accumulates partials in fp32.
"""

import os

import numpy as np

S = 2048
D = 1024
DH = 64
B = 2
NCORES = 8
HPC = 4  # heads per core
QKC = 2 * HPC * DH  # 512 q+k projection columns per core
VC = HPC * DH  # 256 v columns per core
P = 128
KO = D // P  # 8 contraction tiles
NQ = S // 512  # 4 q-chunks of 512
NSC = S // P  # 16 s-chunks of 128

_cache = {}


def _np_compute_dt():
    import ml_dtypes

    return ml_dtypes.bfloat16


def _build():
    import concourse.bacc as bacc
    import concourse.mybir as mybir
    import concourse.tile as tile

    F32 = mybir.dt.float32
    CDT = mybir.dt.bfloat16
    EXP = mybir.ActivationFunctionType.Exp

    nc = bacc.Bacc()
    # xT pre-chunked on host: [4 s-chunks, D, 512]
    xT_d = nc.dram_tensor("xT", [NQ, D, 512], CDT, kind="ExternalInput")
    wqkT_d = nc.dram_tensor("wqkT", [D, QKC], CDT, kind="ExternalInput")
    wvT_d = nc.dram_tensor("wvT", [D, VC], CDT, kind="ExternalInput")
    woT_d = nc.dram_tensor("woT", [VC, D], CDT, kind="ExternalInput")
    maskT_d = nc.dram_tensor("maskT", [P, P], CDT, kind="ExternalInput")
    out_d = nc.dram_tensor("out", [S, D], CDT, kind="ExternalOutput")

    with tile.TileContext(nc) as tc:
        with (
            tc.tile_pool(name="persist", bufs=1) as persist,
            tc.tile_pool(name="sb_small", bufs=4) as sb_small,
            tc.tile_pool(name="sb_exp", bufs=12) as sb_exp,
            tc.tile_pool(name="sb_out", bufs=3) as sb_out,
            tc.tile_pool(name="pp_big", bufs=2, space="PSUM") as pp_big,
            tc.tile_pool(name="pp_av", bufs=2, space="PSUM") as pp_av,
            tc.tile_pool(name="pp_o", bufs=2, space="PSUM") as pp_o,
        ):
            xT_sb = persist.tile([P, KO, S], CDT, tag="xT")
            wqkT_sb = persist.tile([P, KO, QKC], CDT, tag="wqkT")
            wvT_sb = persist.tile([P, KO, VC], CDT, tag="wvT")
            woT_sb = persist.tile([P, 2, D], CDT, tag="woT")
            maskT_sb = persist.tile([P, P], CDT, tag="maskT")
            qkT_sb = persist.tile([P, 4, S], CDT, tag="qkT")
            v_sb = persist.tile([P, NSC, HPC, DH + 1], CDT, tag="v")
            attn_sb = persist.tile([P, 2, S], CDT, tag="attn")

            # --- input DMAs: (wqkT[ko], xT[ko, qb0]) pairs first so the
            # qc=0 projection chains complete after ~2MB of traffic; then
            # maskT + wvT (unblock attention/v-proj qc0), then the
            # remaining xT s-chunks, woT last (needed only by outproj). ---
            for ko in range(KO):
                e1, e2 = (nc.sync, nc.gpsimd) if ko % 2 == 0 else (nc.gpsimd, nc.sync)
                e1.dma_start(wqkT_sb[:, ko, :], wqkT_d[ko * P : (ko + 1) * P, :])
                e2.dma_start(
                    xT_sb[:, ko, 0:512], xT_d[0, ko * P : (ko + 1) * P, :]
                )
            nc.sync.dma_start(maskT_sb[:], maskT_d[:])
            for ko in range(KO):
                e1 = nc.sync if ko % 2 == 0 else nc.gpsimd
                e1.dma_start(wvT_sb[:, ko, :], wvT_d[ko * P : (ko + 1) * P, :])
            for qb in range(1, NQ):
                for ko in range(KO):
                    e1 = nc.sync if ko % 2 == 0 else nc.gpsimd
                    e1.dma_start(
                        xT_sb[:, ko, qb * 512 : (qb + 1) * 512],
                        xT_d[qb, ko * P : (ko + 1) * P, :],
                    )
            nc.gpsimd.dma_start(woT_sb[:], woT_d.rearrange("(ct p) e -> p ct e", p=P))
            ones_f32 = persist.tile([P, DH], F32, tag="ones_f32")
            nc.vector.memset(ones_f32[:], 1.0)
            nc.vector.tensor_copy(
                out=v_sb[:, :, :, DH],
                in_=ones_f32[:, 0 : NSC * HPC].rearrange("p (a b) -> p a b", a=NSC),
            )

            def emit_outproj(qc):
                for si in range(4):
                    sc = qc * 4 + si
                    for en in range(2):
                        ps_o = pp_o.tile([P, 512], F32, tag="o")
                        for ct in range(2):
                            nc.tensor.matmul(
                                ps_o[:],
                                attn_sb[:, ct, sc * P : (sc + 1) * P],
                                woT_sb[:, ct, en * 512 : (en + 1) * 512],
                                start=(ct == 0),
                                stop=(ct == 1),
                                skip_group_check=True,
                            )
                        o_sb = sb_out.tile([P, 512], CDT, tag="osb")
                        nc.vector.tensor_copy(out=o_sb[:], in_=ps_o[:])
                        nc.sync.dma_start(
                            out_d[sc * P : (sc + 1) * P, en * 512 : (en + 1) * 512],
                            o_sb[:],
                        )

            for qc in range(NQ):
                # --- qk projection chunk: pjQ holds all-heads Q (blocks
                # 0,1), pjK all-heads K (blocks 2,3); ko-outer so PE tracks
                # DMA arrival ---
                pjQ = pp_big.tile([P, 1024], F32, tag="big", name="pjQ")
                pjK = pp_big.tile([P, 1024], F32, tag="big", name="pjK")
                for ko in range(KO):
                    for mm in range(4):
                        slot = pjQ if mm < 2 else pjK
                        nc.tensor.matmul(
                            slot[:, (mm % 2) * 512 : (mm % 2 + 1) * 512],
                            wqkT_sb[:, ko, mm * P : (mm + 1) * P],
                            xT_sb[:, ko, qc * 512 : (qc + 1) * 512],
                            start=(ko == 0),
                            stop=(ko == KO - 1),
                            skip_group_check=True,
                        )
                nc.vector.tensor_copy(
                    out=qkT_sb[:, 0:2, qc * 512 : (qc + 1) * 512],
                    in_=pjQ.rearrange("p (a b) -> p a b", a=2),
                )
                nc.vector.tensor_copy(
                    out=qkT_sb[:, 2:4, qc * 512 : (qc + 1) * 512],
                    in_=pjK.rearrange("p (a b) -> p a b", a=2),
                )

                # --- v projection for s-chunks 4qc..4qc+3 (4 bank chains) ---
                pvA = pp_big.tile([P, 1024], F32, tag="big", name="pvA")
                pvB = pp_big.tile([P, 1024], F32, tag="big", name="pvB")
                for ko in range(KO):
                    for j in range(4):
                        slot = pvA if j < 2 else pvB
                        sc = 4 * qc + j
                        nc.tensor.matmul(
                            slot[:, (j % 2) * 512 : (j % 2) * 512 + VC],
                            xT_sb[:, ko, sc * P : (sc + 1) * P],
                            wvT_sb[:, ko, :],
                            start=(ko == 0),
                            stop=(ko == KO - 1),
                            skip_group_check=True,
                        )
                for half, slot in ((0, pvA), (1, pvB)):
                    nc.vector.tensor_copy(
                        out=v_sb[:, 4 * qc + 2 * half : 4 * qc + 2 * half + 2, :, 0:DH],
                        in_=slot.rearrange("p (a h d) -> p a h d", a=2, h=8)[:, :, 0:HPC, :],
                    )

                # --- attention for q-chunk qc: two head-pair passes ---
                nkb = 4 * qc + 4  # causal: k blocks 0 .. 4qc+3
                for p in range(2):  # pair p covers heads (2p, 2p+1)
                    h0, h1 = 2 * p, 2 * p + 1
                    ps_avA = pp_av.tile([DH + 1, 512], F32, tag="av", name="avA")
                    ps_avB = pp_av.tile([DH + 1, 512], F32, tag="av", name="avB")
                    pend = []

                    def flush_av(pend=pend, nkb=nkb, h0=h0, h1=h1,
                                 ps_avA=ps_avA, ps_avB=ps_avB):
                        exp2, off, kb = pend.pop(0)
                        first, last = kb == 0, kb == nkb - 1
                        nc.tensor.matmul(
                            ps_avA[:, off:512],
                            v_sb[:, kb, h0, :],
                            exp2[:, off:512],
                            start=first,
                            stop=last,
                            skip_group_check=True,
                        )
                        nc.tensor.matmul(
                            ps_avB[:, off:512],
                            v_sb[:, kb, h1, :],
                            exp2[:, 512 + off : 1024],
                            start=first,
                            stop=last,
                            skip_group_check=True,
                        )

                    for kb in range(nkb):
                        m = kb - 4 * qc  # >= 0 on diagonal straddlers
                        off = max(0, m) * P
                        ps2 = pp_big.tile([P, 1024], F32, tag="big", name="ps2")
                        exp2 = sb_exp.tile([P, 1024], CDT, tag="exp")
                        # row-tiled head-pair QK^T: h0 on partitions 0-63
                        # (tile (0,0)), h1 on 64-127 (tile (64,0)) - issued
                        # back-to-back so they run concurrently in the PE.
                        nc.tensor.matmul(
                            ps2[:, off:512],
                            qkT_sb[0:DH, 2 + p, kb * P : (kb + 1) * P],
                            qkT_sb[0:DH, p, qc * 512 + off : (qc + 1) * 512],
                            start=True,
                            stop=True,
                            skip_group_check=True,
                        )
                        nc.tensor.matmul(
                            ps2[:, 512 + off : 1024],
                            qkT_sb[DH:P, 2 + p, kb * P : (kb + 1) * P],
                            qkT_sb[DH:P, p, qc * 512 + off : (qc + 1) * 512],
                            start=True,
                            stop=True,
                            skip_group_check=True,
                        )
                        if off == 0:
                            nc.scalar.activation(exp2[:], ps2[:], EXP, scale=0.125)
                        else:
                            nc.scalar.activation(
                                exp2[:, off:512], ps2[:, off:512], EXP, scale=0.125
                            )
                            nc.scalar.activation(
                                exp2[:, 512 + off : 1024],
                                ps2[:, 512 + off : 1024],
                                EXP,
                                scale=0.125,
                            )
                        if m >= 0:
                            nc.vector.tensor_mul(
                                out=exp2[:, off : off + P],
                                in0=exp2[:, off : off + P],
                                in1=maskT_sb[:],
                            )
                            nc.vector.tensor_mul(
                                out=exp2[:, 512 + off : 512 + off + P],
                                in0=exp2[:, 512 + off : 512 + off + P],
                                in1=maskT_sb[:],
                            )
                        pend.append((exp2, off, kb))
                        # lag AV by 2 k-blocks so the PE isn't queued
                        # directly behind an in-flight ACTIVATE
                        if len(pend) > 2:
                            flush_av()
                    while pend:
                        flush_av()

                    # normalize: out = av * (1/sums) broadcast over partitions
                    for h, ps_av in ((h0, ps_avA), (h1, ps_avB)):
                        sums_sb = sb_small.tile([1, 512], F32, tag="sums")
                        nc.vector.tensor_copy(
                            out=sums_sb[:], in_=ps_av[DH : DH + 1, :]
                        )
                        recip_f = sb_small.tile([1, 512], F32, tag="recipf")
                        nc.vector.reciprocal_approx_fast(
                            out=recip_f[:], in_=sums_sb[:]
                        )
                        bc_sb = sb_small.tile([DH, 512], F32, tag="bc")
                        nc.gpsimd.partition_broadcast(bc_sb[:], recip_f[:])
                        hp = (h % 2) * DH
                        nc.vector.tensor_mul(
                            out=attn_sb[hp : hp + DH, h // 2, qc * 512 : (qc + 1) * 512],
                            in0=ps_av[0:DH, :],
                            in1=bc_sb[:],
                        )

                # --- deferred output projection (previous q chunk) ---
                if qc > 0:
                    emit_outproj(qc - 1)
            emit_outproj(NQ - 1)

    nc.compile()
    return nc


def _get_nc():
    if "nc" not in _cache:
        _cache["nc"] = _build()
    return _cache["nc"]


def _shard(x, mask, Wqkv, Wo):
    cdt = _np_compute_dt()
    in_maps = []
    # binary mask for the transposed 128x128 diagonal block:
    # valid (mask==0) -> 1.0, masked (-inf/large-negative) -> 0.0
    maskT = np.ascontiguousarray((mask[0, 0, :P, :P].T >= 0).astype(cdt))
    for c in range(NCORES):
        b = c // 4
        g = c % 4
        heads = [4 * g + i for i in range(HPC)]
        q_rows = np.concatenate([np.arange(h * DH, (h + 1) * DH) for h in heads])
        k_rows = D + q_rows
        v_rows = 2 * D + q_rows
        qk_rows = np.concatenate([q_rows, k_rows])
        xT = x[b].T.astype(cdt)  # [D, S]
        xT_chunks = np.ascontiguousarray(
            xT.reshape(D, NQ, 512).transpose(1, 0, 2)
        )  # [NQ, D, 512]
        in_maps.append(
            {
                "xT": xT_chunks,
                "wqkT": np.ascontiguousarray(Wqkv[qk_rows, :].T.astype(cdt)),
                "wvT": np.ascontiguousarray(Wqkv[v_rows, :].T.astype(cdt)),
                "woT": np.ascontiguousarray(Wo[:, q_rows].T.astype(cdt)),
                "maskT": maskT,
            }
        )
    return in_maps


def kernel(x, mask, Wqkv, Wo, _trace=False):
    from concourse.bass_utils import run_bass_kernel_spmd

    x = np.asarray(x, dtype=np.float32)
    mask = np.asarray(mask, dtype=np.float32)
    Wqkv = np.asarray(Wqkv, dtype=np.float32)
    Wo = np.asarray(Wo, dtype=np.float32)

    nc = _get_nc()
    in_maps = _shard(x, mask, Wqkv, Wo)
    res = run_bass_kernel_spmd(nc, in_maps, core_ids=list(range(NCORES)), trace=_trace)
    _cache["last_result"] = res

    out = np.zeros((B, S, D), dtype=np.float32)
    for c in range(NCORES):
        out[c // 4] += res.results[c]["out"].astype(np.float32)
    return out


# revision 5
# speedup vs baseline: 1.1492x; 1.0531x over previous
"""Causal self-attention Trainium2 kernel (8-core SPMD).

Problem: x[2,2048,1024], causal mask, Wqkv[3072,1024], Wo[1024,1024], fp32.
  qkv = x @ Wqkv.T ; per-head causal softmax attention ; out = attn @ Wo.T

Sharding (data + tensor parallel, per the head dimension):
  core c -> batch b = c // 4, heads {4g..4g+3} with g = c % 4.
  Each core computes Q,K,V for its 4 heads (512 qk cols + 256 v cols of the
  projection), runs causal attention for those heads, and multiplies by the
  matching 256 columns of Wo, producing a partial [2048, 1024] output (bf16).
  Host sums the 4 partials per batch (the tensor-parallel reduction).

Kernel structure (per core):
  - bf16 matmul operands (PE 1 cyc/row), fp32 PSUM accumulation.
  - Projection chunks ko-outer so PE tracks DMA arrival; input DMA is ordered
    (wqkT[ko] + xT[ko, s-chunk0]) pairs first so the first q-chunk's
    projection completes after ~2MB instead of ~5MB of input traffic.
  - Attention per q-chunk runs in two head-pair passes. Scores are computed
    TRANSPOSED (scoresT[k, q]); the two heads of a pair occupy disjoint
    partition halves of qkT (hp 0 / 64), so their QK^T matmuls are issued
    back-to-back and run CONCURRENTLY in the PE via row tiling
    (tile_position (0,0) / (64,0)).
  - Head-pair score blocks land in one 2-bank PSUM tile ([h_even | h_odd]
    512 cols each) so one ACTIVATE exps 1024 columns; the diagonal-straddler
    case exps both valid sub-ranges with ONE strided 3D-AP ACTIVATE.
  - AV is col-tiled: head_even's V[128,64] targets PE columns 0-63, head_odd
    columns 64-127, writing the two partition halves of one PSUM bank
    concurrently. A second col-tiled pair of ones[128,64] matmuls
    accumulates the softmax denominators for both heads into another bank,
    replicated across the same partition halves - so normalization is one
    PSUM copy + one reciprocal + two partition-aligned multiplies, with no
    cross-partition broadcast at all.
  - Shared-PSUM-bank accumulation discipline: the head_even matmul of
    k-block 0 is the only start=True (clears the bank's has_written bits);
    every other matmul accumulates (bits set) or overwrites (bits clear),
    which yields the correct sum in either case.
  - Causality: strictly-upper blocks skipped; diagonal straddlers compute
    only the valid columns; the 128x128 diagonal sub-block is exp'd
    unmasked then multiplied by a binary mask tile (from the mask input).
  - AV/denominator matmuls are emitted lagging the exp pipeline by 2
    k-blocks so the PE never queues directly behind an in-flight ACTIVATE.
  - Output partials are cast to bf16 and DMA'd out (queues alternated);
    the final chunk's casts alternate Scalar/Vector engines since ACT is
    idle by then. The host accumulates partials in fp32.
"""

import os

import numpy as np

S = 2048
D = 1024
DH = 64
B = 2
NCORES = 8
HPC = 4  # heads per core
QKC = 2 * HPC * DH  # 512 q+k projection columns per core
VC = HPC * DH  # 256 v columns per core
P = 128
KO = D // P  # 8 contraction tiles
NQ = S // 512  # 4 q-chunks of 512
NSC = S // P  # 16 s-chunks of 128

_cache = {}


def _np_compute_dt():
    import ml_dtypes

    return ml_dtypes.bfloat16


def _build():
    import concourse.bacc as bacc
    import concourse.mybir as mybir
    import concourse.tile as tile

    F32 = mybir.dt.float32
    CDT = mybir.dt.bfloat16
    EXP = mybir.ActivationFunctionType.Exp
    COPYF = mybir.ActivationFunctionType.Copy

    nc = bacc.Bacc()
    # xT pre-chunked on host: [4 s-chunks, D, 512]
    xT_d = nc.dram_tensor("xT", [NQ, D, 512], CDT, kind="ExternalInput")
    wqkT_d = nc.dram_tensor("wqkT", [D, QKC], CDT, kind="ExternalInput")
    wvT_d = nc.dram_tensor("wvT", [D, VC], CDT, kind="ExternalInput")
    woT_d = nc.dram_tensor("woT", [VC, D], CDT, kind="ExternalInput")
    maskT_d = nc.dram_tensor("maskT", [P, P], CDT, kind="ExternalInput")
    out_d = nc.dram_tensor("out", [S, D], CDT, kind="ExternalOutput")

    with tile.TileContext(nc) as tc:
        with (
            tc.tile_pool(name="persist", bufs=1) as persist,
            tc.tile_pool(name="sb_small", bufs=4) as sb_small,
            tc.tile_pool(name="sb_exp", bufs=12) as sb_exp,
            tc.tile_pool(name="sb_out", bufs=3) as sb_out,
            tc.tile_pool(name="pp_big", bufs=2, space="PSUM") as pp_big,
            tc.tile_pool(name="pp_av", bufs=2, space="PSUM") as pp_av,
            tc.tile_pool(name="pp_o", bufs=2, space="PSUM") as pp_o,
        ):
            xT_sb = persist.tile([P, KO, S], CDT, tag="xT")
            wqkT_sb = persist.tile([P, KO, QKC], CDT, tag="wqkT")
            wvT_sb = persist.tile([P, KO, VC], CDT, tag="wvT")
            woT_sb = persist.tile([P, 2, D], CDT, tag="woT")
            maskT_sb = persist.tile([P, P], CDT, tag="maskT")
            qkT_sb = persist.tile([P, 4, S], CDT, tag="qkT")
            v_sb = persist.tile([P, NSC, HPC, DH], CDT, tag="v")
            attn_sb = persist.tile([P, 2, S], CDT, tag="attn")
            ones64 = persist.tile([P, DH], CDT, tag="ones64")
            nc.vector.memset(ones64[:], 1.0)

            # --- input DMAs: (wqkT[ko], xT[ko, qb0]) pairs first so the
            # qc=0 projection chains complete after ~2MB of traffic; then
            # maskT + wvT (unblock attention/v-proj qc0), then the
            # remaining xT s-chunks, woT last (needed only by outproj). ---
            for ko in range(KO):
                e1, e2 = (nc.sync, nc.gpsimd) if ko % 2 == 0 else (nc.gpsimd, nc.sync)
                e1.dma_start(wqkT_sb[:, ko, :], wqkT_d[ko * P : (ko + 1) * P, :])
                e2.dma_start(
                    xT_sb[:, ko, 0:512], xT_d[0, ko * P : (ko + 1) * P, :]
                )
            nc.sync.dma_start(maskT_sb[:], maskT_d[:])
            for ko in range(KO):
                e1 = nc.sync if ko % 2 == 0 else nc.gpsimd
                e1.dma_start(wvT_sb[:, ko, :], wvT_d[ko * P : (ko + 1) * P, :])
            for qb in range(1, NQ):
                for ko in range(KO):
                    e1 = nc.sync if ko % 2 == 0 else nc.gpsimd
                    e1.dma_start(
                        xT_sb[:, ko, qb * 512 : (qb + 1) * 512],
                        xT_d[qb, ko * P : (ko + 1) * P, :],
                    )
            nc.gpsimd.dma_start(woT_sb[:], woT_d.rearrange("(ct p) e -> p ct e", p=P))

            def emit_outproj(qc, final=False):
                for si in range(4):
                    sc = qc * 4 + si
                    for en in range(2):
                        u = si * 2 + en
                        ps_o = pp_o.tile([P, 512], F32, tag="o")
                        for ct in range(2):
                            nc.tensor.matmul(
                                ps_o[:],
                                attn_sb[:, ct, sc * P : (sc + 1) * P],
                                woT_sb[:, ct, en * 512 : (en + 1) * 512],
                                start=(ct == 0),
                                stop=(ct == 1),
                                skip_group_check=True,
                            )
                        o_sb = sb_out.tile([P, 512], CDT, tag="osb")
                        if final and u % 2 == 0:
                            nc.scalar.activation(o_sb[:], ps_o[:], COPYF)
                        else:
                            nc.vector.tensor_copy(out=o_sb[:], in_=ps_o[:])
                        eng = nc.sync if u % 2 == 0 else nc.gpsimd
                        eng.dma_start(
                            out_d[sc * P : (sc + 1) * P, en * 512 : (en + 1) * 512],
                            o_sb[:],
                        )

            for qc in range(NQ):
                # --- qk projection chunk: pjQ holds all-heads Q (blocks
                # 0,1), pjK all-heads K (blocks 2,3); ko-outer so PE tracks
                # DMA arrival ---
                pjQ = pp_big.tile([P, 1024], F32, tag="big", name="pjQ")
                pjK = pp_big.tile([P, 1024], F32, tag="big", name="pjK")
                for ko in range(KO):
                    for mm in range(4):
                        slot = pjQ if mm < 2 else pjK
                        nc.tensor.matmul(
                            slot[:, (mm % 2) * 512 : (mm % 2 + 1) * 512],
                            wqkT_sb[:, ko, mm * P : (mm + 1) * P],
                            xT_sb[:, ko, qc * 512 : (qc + 1) * 512],
                            start=(ko == 0),
                            stop=(ko == KO - 1),
                            skip_group_check=True,
                        )
                nc.vector.tensor_copy(
                    out=qkT_sb[:, 0:2, qc * 512 : (qc + 1) * 512],
                    in_=pjQ.rearrange("p (a b) -> p a b", a=2),
                )
                nc.vector.tensor_copy(
                    out=qkT_sb[:, 2:4, qc * 512 : (qc + 1) * 512],
                    in_=pjK.rearrange("p (a b) -> p a b", a=2),
                )

                # --- v projection for s-chunks 4qc..4qc+3 (4 bank chains) ---
                pvA = pp_big.tile([P, 1024], F32, tag="big", name="pvA")
                pvB = pp_big.tile([P, 1024], F32, tag="big", name="pvB")
                for ko in range(KO):
                    for j in range(4):
                        slot = pvA if j < 2 else pvB
                        sc = 4 * qc + j
                        nc.tensor.matmul(
                            slot[:, (j % 2) * 512 : (j % 2) * 512 + VC],
                            xT_sb[:, ko, sc * P : (sc + 1) * P],
                            wvT_sb[:, ko, :],
                            start=(ko == 0),
                            stop=(ko == KO - 1),
                            skip_group_check=True,
                        )
                for half, slot in ((0, pvA), (1, pvB)):
                    nc.vector.tensor_copy(
                        out=v_sb[:, 4 * qc + 2 * half : 4 * qc + 2 * half + 2, :, :],
                        in_=slot.rearrange("p (a h d) -> p a h d", a=2, h=8)[:, :, 0:HPC, :],
                    )

                # --- attention for q-chunk qc: two head-pair passes ---
                nkb = 4 * qc + 4  # causal: k blocks 0 .. 4qc+3
                for p in range(2):  # pair p covers heads (2p, 2p+1)
                    h0, h1 = 2 * p, 2 * p + 1
                    # pair_ps: raw AV for h0 on partitions 0-63, h1 on
                    # 64-127 (one shared bank). den_ps: denominators for
                    # h0 / h1 replicated on the same partition halves.
                    pair_ps = pp_av.tile([P, 512], F32, tag="av", name="pair")
                    den_ps = pp_av.tile([P, 512], F32, tag="av", name="den")
                    # zero the shared banks; every matmul below uses
                    # start=False, so each element either accumulates onto
                    # the memset zero (has_written set) or overwrites the
                    # zero with its value (bit clear) - correct in both
                    # cases and independent of matmul execution order.
                    nc.vector.memset(pair_ps[:], 0.0)
                    nc.vector.memset(den_ps[:], 0.0)
                    pend = []

                    def flush_av(pend=pend, nkb=nkb, h0=h0, h1=h1,
                                 pair_ps=pair_ps, den_ps=den_ps):
                        exp2, off, kb = pend.pop(0)
                        last = kb == nkb - 1
                        # col-tiled AV pair: h0 -> PE cols/partitions 0-63,
                        # h1 -> 64-127, concurrent.
                        nc.tensor.matmul(
                            pair_ps[0:DH, off:512],
                            v_sb[:, kb, h0, :],
                            exp2[:, off:512],
                            start=False,
                            stop=last,
                            skip_group_check=True,
                        )
                        nc.tensor.matmul(
                            pair_ps[DH:P, off:512],
                            v_sb[:, kb, h1, :],
                            exp2[:, 512 + off : 1024],
                            start=False,
                            stop=last,
                            skip_group_check=True,
                        )
                        # col-tiled denominator pair (ones stationary)
                        nc.tensor.matmul(
                            den_ps[0:DH, off:512],
                            ones64[:],
                            exp2[:, off:512],
                            start=False,
                            stop=last,
                            skip_group_check=True,
                        )
                        nc.tensor.matmul(
                            den_ps[DH:P, off:512],
                            ones64[:],
                            exp2[:, 512 + off : 1024],
                            start=False,
                            stop=last,
                            skip_group_check=True,
                        )

                    for kb in range(nkb):
                        m = kb - 4 * qc  # >= 0 on diagonal straddlers
                        off = max(0, m) * P
                        ps2 = pp_big.tile([P, 1024], F32, tag="big", name="ps2")
                        exp2 = sb_exp.tile([P, 1024], CDT, tag="exp")
                        # row-tiled head-pair QK^T: h0 on partitions 0-63
                        # (tile (0,0)), h1 on 64-127 (tile (64,0)) - issued
                        # back-to-back so they run concurrently in the PE.
                        nc.tensor.matmul(
                            ps2[:, off:512],
                            qkT_sb[0:DH, 2 + p, kb * P : (kb + 1) * P],
                            qkT_sb[0:DH, p, qc * 512 + off : (qc + 1) * 512],
                            start=True,
                            stop=True,
                            skip_group_check=True,
                        )
                        nc.tensor.matmul(
                            ps2[:, 512 + off : 1024],
                            qkT_sb[DH:P, 2 + p, kb * P : (kb + 1) * P],
                            qkT_sb[DH:P, p, qc * 512 + off : (qc + 1) * 512],
                            start=True,
                            stop=True,
                            skip_group_check=True,
                        )
                        if off == 0:
                            nc.scalar.activation(exp2[:], ps2[:], EXP, scale=0.125)
                        else:
                            # one strided ACTIVATE over both heads' valid
                            # column ranges
                            nc.scalar.activation(
                                exp2.rearrange("p (a b) -> p a b", a=2)[:, :, off:512],
                                ps2.rearrange("p (a b) -> p a b", a=2)[:, :, off:512],
                                EXP,
                                scale=0.125,
                            )
                        if m >= 0:
                            nc.vector.tensor_mul(
                                out=exp2[:, off : off + P],
                                in0=exp2[:, off : off + P],
                                in1=maskT_sb[:],
                            )
                            nc.vector.tensor_mul(
                                out=exp2[:, 512 + off : 512 + off + P],
                                in0=exp2[:, 512 + off : 512 + off + P],
                                in1=maskT_sb[:],
                            )
                        pend.append((exp2, off, kb))
                        # lag AV by 2 k-blocks so the PE isn't queued
                        # directly behind an in-flight ACTIVATE
                        if len(pend) > 2:
                            flush_av()
                    while pend:
                        flush_av()

                    # normalize: one PSUM->SBUF copy of the denominators,
                    # one reciprocal, two partition-aligned multiplies.
                    den_sb = sb_small.tile([P, 512], F32, tag="den")
                    nc.vector.tensor_copy(out=den_sb[:], in_=den_ps[:])
                    recip_sb = sb_small.tile([P, 512], F32, tag="recip")
                    nc.vector.reciprocal_approx_fast(out=recip_sb[:], in_=den_sb[:])
                    for h in (h0, h1):
                        hp = (h % 2) * DH
                        nc.vector.tensor_mul(
                            out=attn_sb[hp : hp + DH, p, qc * 512 : (qc + 1) * 512],
                            in0=pair_ps[hp : hp + DH, :],
                            in1=recip_sb[hp : hp + DH, :],
                        )

                # --- deferred output projection (previous q chunk) ---
                if qc > 0:
                    emit_outproj(qc - 1)
            emit_outproj(NQ - 1, final=True)

    nc.compile()
    return nc


def _get_nc():
    if "nc" not in _cache:
        _cache["nc"] = _build()
    return _cache["nc"]


def _shard(x, mask, Wqkv, Wo):
    cdt = _np_compute_dt()
    in_maps = []
    # binary mask for the transposed 128x128 diagonal block:
    # valid (mask==0) -> 1.0, masked (-inf/large-negative) -> 0.0
    maskT = np.ascontiguousarray((mask[0, 0, :P, :P].T >= 0).astype(cdt))
    for c in range(NCORES):
        b = c // 4
        g = c % 4
        heads = [4 * g + i for i in range(HPC)]
        q_rows = np.concatenate([np.arange(h * DH, (h + 1) * DH) for h in heads])
        k_rows = D + q_rows
        v_rows = 2 * D + q_rows
        qk_rows = np.concatenate([q_rows, k_rows])
        xT = x[b].T.astype(cdt)  # [D, S]
        xT_chunks = np.ascontiguousarray(
            xT.reshape(D, NQ, 512).transpose(1, 0, 2)
        )  # [NQ, D, 512]
        in_maps.append(
            {
                "xT": xT_chunks,
                "wqkT": np.ascontiguousarray(Wqkv[qk_rows, :].T.astype(cdt)),
                "wvT": np.ascontiguousarray(Wqkv[v_rows, :].T.astype(cdt)),
                "woT": np.ascontiguousarray(Wo[:, q_rows].T.astype(cdt)),
                "maskT": maskT,
            }
        )
    return in_maps


def kernel(x, mask, Wqkv, Wo, _trace=False):
    from concourse.bass_utils import run_bass_kernel_spmd

    x = np.asarray(x, dtype=np.float32)
    mask = np.asarray(mask, dtype=np.float32)
    Wqkv = np.asarray(Wqkv, dtype=np.float32)
    Wo = np.asarray(Wo, dtype=np.float32)

    nc = _get_nc()
    in_maps = _shard(x, mask, Wqkv, Wo)
    res = run_bass_kernel_spmd(nc, in_maps, core_ids=list(range(NCORES)), trace=_trace)
    _cache["last_result"] = res

    out = np.zeros((B, S, D), dtype=np.float32)
    for c in range(NCORES):
        out[c // 4] += res.results[c]["out"].astype(np.float32)
    return out
